# revision 25
# baseline (speedup 1.0000x reference)
"""Trainium2 Bass kernel for the tiny-MLP ensemble problem.

Computes, for B collocation points x[B,3]:
    u[b, n, k] = tanh(sum_d x[b,d] * W[n,k,d] + b[n,k])     (N=64 layers, 3x3 each)
    out_j[b]   = sum_n c_j[n] * u[b, n, j]                  (j in {rho, p, u})

Strategy (pure data parallel over 8 cores, B/8 points each):
  - Host packs x (transposed + grouped, fp16) so the device sees a stacked
    layout: 32 point-groups x 4 rows (x0, x1, x2, 1.0) = 128 SBUF partitions;
    columns are points. The constant-1 row folds the bias into the matmul.
  - MM1 (bf16): block-diagonal stationary [128,128] computes 4 of the 192
    tanh-inputs for each of the 32 groups at once -> PSUM [128, 512] per
    slice (48 slices).
  - ScalarE tanh reads PSUM directly (batched slices per instruction),
    writes bf16 to SBUF.
  - MM2 (bf16): block-diagonal stationary [128,96] contracts the 4
    nk-channels per slice against the head vectors, accumulating all 48
    slices into one PSUM [96, 512] tile (rows = 3 heads x 32 groups).
  - VectorE copies the accumulated heads to SBUF; DMA out; host un-permutes.
"""

import numpy as np

import concourse.bacc as bacc
import concourse.bass as bass
import concourse.mybir as mybir
import concourse.tile as tile
from concourse.bass_utils import run_bass_kernel_spmd

F32 = mybir.dt.float32
F16 = mybir.dt.float16
NP_F16 = np.float16

N_CORES = 8
B_FULL = 1_000_000
NL, D = 64, 3
NK = NL * D            # 192 tanh channels per point
G = 32                 # point groups stacked on partitions
CW = 512               # points per group per supergroup
KOUT = 4               # nk channels per group per slice
NSLICES = NK // KOUT   # 48
SPA = 3                # slices batched per activation instruction (3 PSUM banks)
PER_CORE_RAW = B_FULL // N_CORES      # 125000
# Per-supergroup column widths: 7 full 512-wide SGs plus a 323-wide tail so
# the padded per-core point count is 32*3907 = 125024 (0.02% waste).
WIDTHS = [CW] * 7 + [323]
COLS = sum(WIDTHS)                    # 3907 device columns per core
SG = len(WIDTHS)
PER_CORE = G * COLS                   # 125024 (padded)
COL_OFF = [sum(WIDTHS[:i]) for i in range(SG)]

_NC_CACHE = {}

PROFILE = False
LAST_RESULT = None

# DVE tanh offload: slice groups handled by VectorE instead of ScalarE.
# tanh(x) ~= q0(y) * q1(y) * q2(y) * x, y = x^2 clamped to [0, P_C^2], where
# q0 = P_K0*y + P_K0B, qi = (sqrt(Si)*(y+hi))^2 + Si*di (translated squares,
# all-positive -> no f16 cancellation). Max |err| vs tanh ~= 5.9e-3.
OFFLOAD_M = (15,)
P_C = 3.3984375
_c5, _r0 = -1.3752515942499854e-05, 15.125242546284024
_a1, _b1 = -20.345252650120308, 143.7371328484062
_a2, _b2 = -2.1770791873739466, 32.851306410241875
_S0, _S1, _S2 = 2048.0, 1 / 32.0, 1 / 64.0
P_K0, P_K0B = _S0 * _c5, -_S0 * _c5 * _r0
P_KS1, P_KH1 = _S1 ** 0.5, _S1 ** 0.5 * (_a1 / 2)
P_KS2, P_KH2 = _S2 ** 0.5, _S2 ** 0.5 * (_a2 / 2)
P_KD1 = _S1 * (_b1 - _a1 * _a1 / 4)
P_KD2 = _S2 * (_b2 - _a2 * _a2 / 4)


def build_nc():
    nc = bacc.Bacc("TRN2", target_bir_lowering=False, debug=False,
                   num_devices=N_CORES)
    x4 = nc.dram_tensor("x4", [4 * G, COLS], F16, kind="ExternalInput")
    w1 = nc.dram_tensor("w1", [4 * G, NSLICES * 128], F16, kind="ExternalInput")
    w2 = nc.dram_tensor("w2", [128, NSLICES * 3 * G], F16, kind="ExternalInput")
    out = nc.dram_tensor("out", [3 * G, COLS], F16, kind="ExternalOutput")

    with tile.TileContext(nc) as tc:
        with (
            tc.tile_pool(name="wpool", bufs=1) as wpool,
            tc.tile_pool(name="xpool", bufs=3) as xpool,
            tc.tile_pool(name="upool", bufs=3) as upool,
            tc.tile_pool(name="opool", bufs=2) as opool,
            tc.tile_pool(name="vpool", bufs=2) as vpool,
            tc.tile_pool(name="ofpool", bufs=2) as ofpool,
            tc.tile_pool(name="pu", bufs=2, space=bass.MemorySpace.PSUM) as pupool,
            tc.tile_pool(name="po", bufs=2, space=bass.MemorySpace.PSUM) as popool,
        ):
            w1_sb = wpool.tile([4 * G, NSLICES * 128], F16, tag="w1")
            w2_sb = wpool.tile([128, NSLICES * 3 * G], F16, tag="w2")

            # First x chunk goes out on the sync queue ahead of everything;
            # weights stream in per-m-group chunks on the otherwise-idle
            # vector/gpsimd DMA queues so the first matmul can start as soon
            # as x chunk 0 plus weight chunk 0 have landed.
            xts = {}
            xts[0] = xpool.tile([4 * G, WIDTHS[0]], F16, name="xt0")
            nc.sync.dma_start(out=xts[0][:], in_=x4[:, 0:WIDTHS[0]])
            NM = NSLICES // SPA
            WCH = 8  # weight DMA chunks (6 slices each)
            SL = NSLICES // WCH

            def w1ch(ch):
                return (ch * SL * 128, (ch + 1) * SL * 128)

            def w2ch(ch):
                return (ch * SL * 3 * G, (ch + 1) * SL * 3 * G)

            # chunk 0 of each weight rides the fast sync hwdge queue so the
            # first slice group can start ~1.5us in; the rest stream on
            # gpsimd's software-DGE queue, interleaved w1/w2 so neither
            # starves.
            c0, c1 = w1ch(0)
            nc.sync.dma_start(out=w1_sb[:, c0:c1], in_=w1[:, c0:c1])
            c0, c1 = w2ch(0)
            nc.sync.dma_start(out=w2_sb[:, c0:c1], in_=w2[:, c0:c1])
            for ch in range(1, WCH):
                c0, c1 = w1ch(ch)
                nc.gpsimd.dma_start(out=w1_sb[:, c0:c1], in_=w1[:, c0:c1])
                c0, c1 = w2ch(ch)
                nc.gpsimd.dma_start(out=w2_sb[:, c0:c1], in_=w2[:, c0:c1])

            def deferred_mm2(pend, q_):
                po_, (m_, ut_), w_, o0_ = pend
                s_ = m_ * SPA + q_
                nc.tensor.matmul(
                    po_[:],
                    w2_sb[:, s_ * 3 * G:(s_ + 1) * 3 * G],
                    ut_[:, q_, :],
                    start=False,
                    stop=(q_ == SPA - 1),
                    skip_group_check=True,
                )

            def sg_out(po_, w_, o0_):
                ot = opool.tile([3 * G, w_], F16)
                nc.vector.tensor_copy(ot[:], po_[:])
                nc.sync.dma_start(out=out[:, o0_:o0_ + w_], in_=ot[:])

            pending = None
            for t in range(SG):
                w = WIDTHS[t]
                o0 = COL_OFF[t]
                if t not in xts:
                    xts[t] = xpool.tile([4 * G, w], F16, name=f"xt{t}")
                    nc.sync.dma_start(out=xts[t][:], in_=x4[:, o0:o0 + w])
                xt = xts[t]
                po = popool.tile([3 * G, w], F32)
                deferred = []
                # Never offload on the final SG: its deferred MM2s would sit
                # at the very end with nothing to hide the DVE chain behind.
                off_m = OFFLOAD_M if t < SG - 1 else ()
                for m in range(NM):
                    # [128, SPA, CW] keeps each matmul's output slice aligned
                    # to a PSUM bank even when w < CW (matmul output must not
                    # cross a bank boundary).
                    pu = pupool.tile([128, SPA, CW], F32)
                    for q in range(SPA):
                        s = m * SPA + q
                        nc.tensor.matmul(
                            pu[:, q, 0:w],
                            w1_sb[:, s * 128:(s + 1) * 128],
                            xt[:],
                            start=True, stop=True,
                        )
                    # Spread the previous SG's deferred MM2s one per m-group,
                    # after this group's MM1s (so ScalarE never waits on the
                    # in-order PE queue), well after the DVE chain finished.
                    if pending is not None and 10 <= m < 10 + SPA:
                        deferred_mm2(pending, m - 10)
                        if m == 10 + SPA - 1:
                            sg_out(pending[0], pending[2], pending[3])
                            pending = None
                    if m in off_m:
                        ut = ofpool.tile([128, SPA, w], F16, name="ut_off")
                    else:
                        ut = upool.tile([128, SPA, w], F16)
                    if m in off_m:
                        # VectorE computes tanh via a factored deg-11 odd
                        # polynomial (translated squares, no cancellation) to
                        # relieve the ScalarE bottleneck for this slice group.
                        A = mybir.AluOpType
                        xc32 = vpool.tile([128, SPA, w], F32)
                        nc.vector.tensor_scalar(
                            xc32[:], pu[:, :, 0:w], P_C, -P_C, A.min, A.max)
                        y32 = vpool.tile([128, SPA, w], F32)
                        nc.vector.tensor_tensor(y32[:], xc32[:], xc32[:], op=A.mult)
                        xc16 = vpool.tile([128, SPA, w], F16)
                        nc.vector.tensor_copy(xc16[:], xc32[:])
                        q0 = vpool.tile([128, SPA, w], F16)
                        nc.vector.tensor_scalar(q0[:], y32[:], P_K0, P_K0B, A.mult, A.add)
                        zs1 = vpool.tile([128, SPA, w], F16)
                        nc.vector.tensor_scalar(zs1[:], y32[:], P_KS1, P_KH1, A.mult, A.add)
                        zs2 = vpool.tile([128, SPA, w], F16)
                        nc.vector.tensor_scalar(zs2[:], y32[:], P_KS2, P_KH2, A.mult, A.add)
                        sq1 = vpool.tile([128, SPA, w], F16)
                        nc.vector.tensor_tensor(sq1[:], zs1[:], zs1[:], op=A.mult)
                        sq2 = vpool.tile([128, SPA, w], F16)
                        nc.vector.tensor_tensor(sq2[:], zs2[:], zs2[:], op=A.mult)
                        q1 = vpool.tile([128, SPA, w], F16)
                        nc.vector.tensor_scalar(q1[:], sq1[:], P_KD1, None, A.add)
                        q2 = vpool.tile([128, SPA, w], F16)
                        nc.vector.tensor_scalar(q2[:], sq2[:], P_KD2, None, A.add)
                        m1 = vpool.tile([128, SPA, w], F16)
                        nc.vector.tensor_tensor(m1[:], q0[:], q1[:], op=A.mult)
                        m2 = vpool.tile([128, SPA, w], F16)
                        nc.vector.tensor_tensor(m2[:], q2[:], xc16[:], op=A.mult)
                        nc.vector.tensor_tensor(ut[:], m1[:], m2[:], op=A.mult)
                        # Defer this group's MM2s to the end of the SG: the PE
                        # executes in program order, so issuing them here would
                        # stall the whole pipeline behind the DVE chain. PSUM
                        # accumulation is order-independent.
                        deferred.append((m, ut))
                        continue
                    else:
                        nc.scalar.activation(
                            ut[:], pu[:, :, 0:w],
                            mybir.ActivationFunctionType.Tanh)
                    for q in range(SPA):
                        s = m * SPA + q
                        nc.tensor.matmul(
                            po[:],
                            w2_sb[:, s * 3 * G:(s + 1) * 3 * G],
                            ut[:, q, :],
                            start=(s == 0), stop=(s == NSLICES - 1),
                            skip_group_check=True,
                        )
                if deferred:
                    pending = (po, deferred[0], w, o0)
                else:
                    sg_out(po, w, o0)
            if pending is not None:
                for q in range(SPA):
                    deferred_mm2(pending, q)
                sg_out(pending[0], pending[2], pending[3])

    nc.compile()
    return nc


def get_nc():
    if 0 not in _NC_CACHE:
        _NC_CACHE[0] = build_nc()
    return _NC_CACHE[0]


def pack_weights(W, b, c_rho, c_p, c_u):
    """Build the two block-diagonal stationary matrices (all slices packed)."""
    W = np.asarray(W, np.float32)
    b = np.asarray(b, np.float32)
    # Wcat[d, 3n+k] = W[n, k, d]
    wcat = np.ascontiguousarray(W.transpose(2, 0, 1)).reshape(3, NK)
    bflat = np.asarray(b, np.float32).reshape(NK)
    # C[3n+j, j] = c_j[n]
    C = np.zeros((NK, 3), np.float32)
    C[np.arange(NL) * 3 + 0, 0] = np.asarray(c_rho, np.float32).reshape(NL)
    C[np.arange(NL) * 3 + 1, 1] = np.asarray(c_p, np.float32).reshape(NL)
    C[np.arange(NL) * 3 + 2, 2] = np.asarray(c_u, np.float32).reshape(NL)

    gi = np.arange(G)
    # w1[(4g+d), s*128 + 4g+o] = Wcat[d, 4s+o]  (d<3);  = bflat[4s+o] (d==3)
    w1 = np.zeros((G, 4, NSLICES, G, KOUT), np.float32)
    w1[gi, :3, :, gi, :] = wcat.reshape(3, NSLICES, KOUT)[None]
    w1[gi, 3, :, gi, :] = bflat.reshape(NSLICES, KOUT)[None]
    w1 = np.ascontiguousarray(w1).reshape(4 * G, NSLICES * 128)

    # w2[(4g+o), s*96 + 3g+j] = C[4s+o, j]
    w2 = np.zeros((G, KOUT, NSLICES, G, 3), np.float32)
    w2[gi, :, :, gi, :] = C.reshape(NSLICES, KOUT, 3).transpose(1, 0, 2)[None]
    w2 = np.ascontiguousarray(w2).reshape(128, NSLICES * 3 * G)
    return w1.astype(NP_F16), w2.astype(NP_F16)


def pack_x_core(x_core_padded):
    """[PER_CORE, 3] -> [128, COLS] stacked layout with constant-1 rows."""
    parts = []
    off = 0
    for w in WIDTHS:
        blk = x_core_padded[off:off + G * w].reshape(G, w, 3)
        x4 = np.ones((G, 4, w), np.float32)
        x4[:, :3] = blk.transpose(0, 2, 1)
        parts.append(x4.reshape(4 * G, w))
        off += G * w
    return np.ascontiguousarray(np.concatenate(parts, 1)).astype(NP_F16)


def unpack_out_core(out_dev):
    """[96, COLS] (f16) -> [PER_CORE, 3] (f32)."""
    parts = []
    for t, w in enumerate(WIDTHS):
        o0 = COL_OFF[t]
        o = out_dev[:, o0:o0 + w].astype(np.float32)
        parts.append(o.reshape(G, 3, w).transpose(0, 2, 1).reshape(G * w, 3))
    return np.concatenate(parts, 0)


def kernel(x, W, b, c_rho, c_p, c_u):
    x = np.asarray(x, np.float32)
    nc = get_nc()
    w1, w2 = pack_weights(W, b, c_rho, c_p, c_u)

    in_maps = []
    for c in range(N_CORES):
        off = c * PER_CORE_RAW
        xc = np.zeros((PER_CORE, 3), np.float32)
        xc[:PER_CORE_RAW] = x[off:off + PER_CORE_RAW]
        in_maps.append({"x4": pack_x_core(xc), "w1": w1, "w2": w2})

    res = run_bass_kernel_spmd(nc, in_maps, list(range(N_CORES)),
                               trace=PROFILE)
    if PROFILE:
        globals()["LAST_RESULT"] = res
    outs = []
    for c in range(N_CORES):
        outs.append(unpack_out_core(res.results[c]["out"])[:PER_CORE_RAW])
    full = np.concatenate(outs, axis=0)  # [1M, 3]
    return (np.ascontiguousarray(full[:, 0:1]),
            np.ascontiguousarray(full[:, 1:2]),
            np.ascontiguousarray(full[:, 2:3]))


# revision 27
# speedup vs baseline: 1.0047x; 1.0047x over previous
"""Trainium2 Bass kernel for the tiny-MLP ensemble problem.

Computes, for B collocation points x[B,3]:
    u[b, n, k] = tanh(sum_d x[b,d] * W[n,k,d] + b[n,k])     (N=64 layers, 3x3 each)
    out_j[b]   = sum_n c_j[n] * u[b, n, j]                  (j in {rho, p, u})

Strategy (pure data parallel over 8 cores, ~B/8 points each, fp16 device
datapath, fp32 PSUM accumulation; ScalarE tanh throughput is the bound):
  - Host packs x (transposed + grouped, fp16) so the device sees a stacked
    layout: 32 point-groups x 4 rows (x0, x1, x2, 1.0) = 128 SBUF partitions;
    columns are points. The constant-1 row folds the bias into the matmul.
    Column widths 7x512 + 323 pad 125000 -> 125024 points/core (0.02%).
  - MM1 (fp16): block-diagonal stationary [128,128] computes 4 of the 192
    tanh-inputs for each of the 32 groups at once -> one PSUM bank per
    slice (48 slices), 3 slices per activation batch.
  - ScalarE tanh reads PSUM directly ([128, 3, w] strided), writes fp16 u
    to SBUF. ~189us/core of ACT time is the kernel's critical path.
  - MM2 (fp16): block-diagonal stationary [128,96] contracts the 4
    nk-channels per slice against the head vectors, accumulating all 48
    slices into one PSUM [96, w] tile (rows = 3 heads x 32 groups).
  - VectorE copies the accumulated heads to SBUF (fp16); DMA out; host
    un-permutes and upcasts.
  - Weights stream in chunks (first on the sync queue, rest via gpsimd) so
    the first matmul starts ~2us in instead of after a 3.7MB bulk load.
  - OFFLOAD_M can route slice groups to a VectorE polynomial tanh; measured
    net-negative on HW (in-order PE queue stalls), so disabled.
"""

import numpy as np

import concourse.bacc as bacc
import concourse.bass as bass
import concourse.mybir as mybir
import concourse.tile as tile
from concourse.bass_utils import run_bass_kernel_spmd

F32 = mybir.dt.float32
F16 = mybir.dt.float16
NP_F16 = np.float16

N_CORES = 8
B_FULL = 1_000_000
NL, D = 64, 3
NK = NL * D            # 192 tanh channels per point
G = 32                 # point groups stacked on partitions
CW = 512               # points per group per supergroup
KOUT = 4               # nk channels per group per slice
NSLICES = NK // KOUT   # 48
SPA = 3                # slices batched per activation instruction (3 PSUM banks)
PER_CORE_RAW = B_FULL // N_CORES      # 125000
# Per-supergroup column widths: 7 full 512-wide SGs plus a 323-wide tail so
# the padded per-core point count is 32*3907 = 125024 (0.02% waste).
WIDTHS = [CW] * 7 + [323]
COLS = sum(WIDTHS)                    # 3907 device columns per core
SG = len(WIDTHS)
PER_CORE = G * COLS                   # 125024 (padded)
COL_OFF = [sum(WIDTHS[:i]) for i in range(SG)]

_NC_CACHE = {}

PROFILE = False
LAST_RESULT = None

# DVE tanh offload: slice groups handled by VectorE instead of ScalarE.
# tanh(x) ~= q0(y) * q1(y) * q2(y) * x, y = x^2 clamped to [0, P_C^2], where
# q0 = P_K0*y + P_K0B, qi = (sqrt(Si)*(y+hi))^2 + Si*di (translated squares,
# all-positive -> no f16 cancellation). Max |err| vs tanh ~= 5.9e-3.
OFFLOAD_M = ()  # measured net-negative on HW (in-order PE stalls eat the ACT win)
P_C = 3.3984375
_c5, _r0 = -1.3752515942499854e-05, 15.125242546284024
_a1, _b1 = -20.345252650120308, 143.7371328484062
_a2, _b2 = -2.1770791873739466, 32.851306410241875
_S0, _S1, _S2 = 2048.0, 1 / 32.0, 1 / 64.0
P_K0, P_K0B = _S0 * _c5, -_S0 * _c5 * _r0
P_KS1, P_KH1 = _S1 ** 0.5, _S1 ** 0.5 * (_a1 / 2)
P_KS2, P_KH2 = _S2 ** 0.5, _S2 ** 0.5 * (_a2 / 2)
P_KD1 = _S1 * (_b1 - _a1 * _a1 / 4)
P_KD2 = _S2 * (_b2 - _a2 * _a2 / 4)


def build_nc():
    nc = bacc.Bacc("TRN2", target_bir_lowering=False, debug=False,
                   num_devices=N_CORES)
    x4 = nc.dram_tensor("x4", [4 * G, COLS], F16, kind="ExternalInput")
    w1 = nc.dram_tensor("w1", [4 * G, NSLICES * 128], F16, kind="ExternalInput")
    w2 = nc.dram_tensor("w2", [128, NSLICES * 3 * G], F16, kind="ExternalInput")
    out = nc.dram_tensor("out", [3 * G, COLS], F16, kind="ExternalOutput")

    with tile.TileContext(nc) as tc:
        with (
            tc.tile_pool(name="wpool", bufs=1) as wpool,
            tc.tile_pool(name="xpool", bufs=3) as xpool,
            tc.tile_pool(name="upool", bufs=3) as upool,
            tc.tile_pool(name="opool", bufs=2) as opool,
            tc.tile_pool(name="vpool", bufs=2) as vpool,
            tc.tile_pool(name="ofpool", bufs=2) as ofpool,
            tc.tile_pool(name="pu", bufs=2, space=bass.MemorySpace.PSUM) as pupool,
            tc.tile_pool(name="po", bufs=2, space=bass.MemorySpace.PSUM) as popool,
        ):
            w1_sb = wpool.tile([4 * G, NSLICES * 128], F16, tag="w1")
            w2_sb = wpool.tile([128, NSLICES * 3 * G], F16, tag="w2")

            # First x chunk goes out on the sync queue ahead of everything;
            # weights stream in per-m-group chunks on the otherwise-idle
            # vector/gpsimd DMA queues so the first matmul can start as soon
            # as x chunk 0 plus weight chunk 0 have landed.
            xts = {}
            xts[0] = xpool.tile([4 * G, WIDTHS[0]], F16, name="xt0")
            nc.sync.dma_start(out=xts[0][:], in_=x4[:, 0:WIDTHS[0]])
            NM = NSLICES // SPA
            WCH = 8  # weight DMA chunks (6 slices each)
            SL = NSLICES // WCH

            def w1ch(ch):
                return (ch * SL * 128, (ch + 1) * SL * 128)

            def w2ch(ch):
                return (ch * SL * 3 * G, (ch + 1) * SL * 3 * G)

            # chunk 0 of each weight rides the fast sync hwdge queue so the
            # first slice group can start ~1.5us in; the rest stream on
            # gpsimd's software-DGE queue, interleaved w1/w2 so neither
            # starves.
            c0, c1 = w1ch(0)
            nc.sync.dma_start(out=w1_sb[:, c0:c1], in_=w1[:, c0:c1])
            c0, c1 = w2ch(0)
            nc.sync.dma_start(out=w2_sb[:, c0:c1], in_=w2[:, c0:c1])
            for ch in range(1, WCH):
                c0, c1 = w1ch(ch)
                nc.gpsimd.dma_start(out=w1_sb[:, c0:c1], in_=w1[:, c0:c1])
                c0, c1 = w2ch(ch)
                nc.gpsimd.dma_start(out=w2_sb[:, c0:c1], in_=w2[:, c0:c1])

            def deferred_mm2(pend, q_):
                po_, (m_, ut_), w_, o0_ = pend
                s_ = m_ * SPA + q_
                nc.tensor.matmul(
                    po_[:],
                    w2_sb[:, s_ * 3 * G:(s_ + 1) * 3 * G],
                    ut_[:, q_, :],
                    start=False,
                    stop=(q_ == SPA - 1),
                    skip_group_check=True,
                )

            def sg_out(po_, w_, o0_):
                ot = opool.tile([3 * G, w_], F16)
                nc.vector.tensor_copy(ot[:], po_[:])
                nc.sync.dma_start(out=out[:, o0_:o0_ + w_], in_=ot[:])

            pending = None
            for t in range(SG):
                w = WIDTHS[t]
                o0 = COL_OFF[t]
                if t not in xts:
                    xts[t] = xpool.tile([4 * G, w], F16, name=f"xt{t}")
                    nc.sync.dma_start(out=xts[t][:], in_=x4[:, o0:o0 + w])
                xt = xts[t]
                po = popool.tile([3 * G, w], F32)
                deferred = []
                # Never offload on the final SG: its deferred MM2s would sit
                # at the very end with nothing to hide the DVE chain behind.
                off_m = OFFLOAD_M if t < SG - 1 else ()
                for m in range(NM):
                    # [128, SPA, CW] keeps each matmul's output slice aligned
                    # to a PSUM bank even when w < CW (matmul output must not
                    # cross a bank boundary).
                    pu = pupool.tile([128, SPA, CW], F32)
                    for q in range(SPA):
                        s = m * SPA + q
                        nc.tensor.matmul(
                            pu[:, q, 0:w],
                            w1_sb[:, s * 128:(s + 1) * 128],
                            xt[:],
                            start=True, stop=True,
                        )
                    # Spread the previous SG's deferred MM2s one per m-group,
                    # after this group's MM1s (so ScalarE never waits on the
                    # in-order PE queue), well after the DVE chain finished.
                    if pending is not None and 10 <= m < 10 + SPA:
                        deferred_mm2(pending, m - 10)
                        if m == 10 + SPA - 1:
                            sg_out(pending[0], pending[2], pending[3])
                            pending = None
                    if m in off_m:
                        ut = ofpool.tile([128, SPA, w], F16, name="ut_off")
                    else:
                        ut = upool.tile([128, SPA, w], F16)
                    if m in off_m:
                        # VectorE computes tanh via a factored deg-11 odd
                        # polynomial (translated squares, no cancellation) to
                        # relieve the ScalarE bottleneck for this slice group.
                        A = mybir.AluOpType
                        xc32 = vpool.tile([128, SPA, w], F32)
                        nc.vector.tensor_scalar(
                            xc32[:], pu[:, :, 0:w], P_C, -P_C, A.min, A.max)
                        y32 = vpool.tile([128, SPA, w], F32)
                        nc.vector.tensor_tensor(y32[:], xc32[:], xc32[:], op=A.mult)
                        xc16 = vpool.tile([128, SPA, w], F16)
                        nc.vector.tensor_copy(xc16[:], xc32[:])
                        q0 = vpool.tile([128, SPA, w], F16)
                        nc.vector.tensor_scalar(q0[:], y32[:], P_K0, P_K0B, A.mult, A.add)
                        zs1 = vpool.tile([128, SPA, w], F16)
                        nc.vector.tensor_scalar(zs1[:], y32[:], P_KS1, P_KH1, A.mult, A.add)
                        zs2 = vpool.tile([128, SPA, w], F16)
                        nc.vector.tensor_scalar(zs2[:], y32[:], P_KS2, P_KH2, A.mult, A.add)
                        sq1 = vpool.tile([128, SPA, w], F16)
                        nc.vector.tensor_tensor(sq1[:], zs1[:], zs1[:], op=A.mult)
                        sq2 = vpool.tile([128, SPA, w], F16)
                        nc.vector.tensor_tensor(sq2[:], zs2[:], zs2[:], op=A.mult)
                        q1 = vpool.tile([128, SPA, w], F16)
                        nc.vector.tensor_scalar(q1[:], sq1[:], P_KD1, None, A.add)
                        q2 = vpool.tile([128, SPA, w], F16)
                        nc.vector.tensor_scalar(q2[:], sq2[:], P_KD2, None, A.add)
                        m1 = vpool.tile([128, SPA, w], F16)
                        nc.vector.tensor_tensor(m1[:], q0[:], q1[:], op=A.mult)
                        m2 = vpool.tile([128, SPA, w], F16)
                        nc.vector.tensor_tensor(m2[:], q2[:], xc16[:], op=A.mult)
                        nc.vector.tensor_tensor(ut[:], m1[:], m2[:], op=A.mult)
                        # Defer this group's MM2s to the end of the SG: the PE
                        # executes in program order, so issuing them here would
                        # stall the whole pipeline behind the DVE chain. PSUM
                        # accumulation is order-independent.
                        deferred.append((m, ut))
                        continue
                    else:
                        nc.scalar.activation(
                            ut[:], pu[:, :, 0:w],
                            mybir.ActivationFunctionType.Tanh)
                    for q in range(SPA):
                        s = m * SPA + q
                        nc.tensor.matmul(
                            po[:],
                            w2_sb[:, s * 3 * G:(s + 1) * 3 * G],
                            ut[:, q, :],
                            start=(s == 0), stop=(s == NSLICES - 1),
                            skip_group_check=True,
                        )
                if deferred:
                    pending = (po, deferred[0], w, o0)
                else:
                    sg_out(po, w, o0)
            if pending is not None:
                for q in range(SPA):
                    deferred_mm2(pending, q)
                sg_out(pending[0], pending[2], pending[3])

    nc.compile()
    return nc


def get_nc():
    if 0 not in _NC_CACHE:
        _NC_CACHE[0] = build_nc()
    return _NC_CACHE[0]


def pack_weights(W, b, c_rho, c_p, c_u):
    """Build the two block-diagonal stationary matrices (all slices packed)."""
    W = np.asarray(W, np.float32)
    b = np.asarray(b, np.float32)
    # Wcat[d, 3n+k] = W[n, k, d]
    wcat = np.ascontiguousarray(W.transpose(2, 0, 1)).reshape(3, NK)
    bflat = np.asarray(b, np.float32).reshape(NK)
    # C[3n+j, j] = c_j[n]
    C = np.zeros((NK, 3), np.float32)
    C[np.arange(NL) * 3 + 0, 0] = np.asarray(c_rho, np.float32).reshape(NL)
    C[np.arange(NL) * 3 + 1, 1] = np.asarray(c_p, np.float32).reshape(NL)
    C[np.arange(NL) * 3 + 2, 2] = np.asarray(c_u, np.float32).reshape(NL)

    gi = np.arange(G)
    # w1[(4g+d), s*128 + 4g+o] = Wcat[d, 4s+o]  (d<3);  = bflat[4s+o] (d==3)
    w1 = np.zeros((G, 4, NSLICES, G, KOUT), np.float32)
    w1[gi, :3, :, gi, :] = wcat.reshape(3, NSLICES, KOUT)[None]
    w1[gi, 3, :, gi, :] = bflat.reshape(NSLICES, KOUT)[None]
    w1 = np.ascontiguousarray(w1).reshape(4 * G, NSLICES * 128)

    # w2[(4g+o), s*96 + 3g+j] = C[4s+o, j]
    w2 = np.zeros((G, KOUT, NSLICES, G, 3), np.float32)
    w2[gi, :, :, gi, :] = C.reshape(NSLICES, KOUT, 3).transpose(1, 0, 2)[None]
    w2 = np.ascontiguousarray(w2).reshape(128, NSLICES * 3 * G)
    return w1.astype(NP_F16), w2.astype(NP_F16)


def pack_x_core(x_core_padded):
    """[PER_CORE, 3] -> [128, COLS] stacked layout with constant-1 rows."""
    parts = []
    off = 0
    for w in WIDTHS:
        blk = x_core_padded[off:off + G * w].reshape(G, w, 3)
        x4 = np.ones((G, 4, w), np.float32)
        x4[:, :3] = blk.transpose(0, 2, 1)
        parts.append(x4.reshape(4 * G, w))
        off += G * w
    return np.ascontiguousarray(np.concatenate(parts, 1)).astype(NP_F16)


def unpack_out_core(out_dev):
    """[96, COLS] (f16) -> [PER_CORE, 3] (f32)."""
    parts = []
    for t, w in enumerate(WIDTHS):
        o0 = COL_OFF[t]
        o = out_dev[:, o0:o0 + w].astype(np.float32)
        parts.append(o.reshape(G, 3, w).transpose(0, 2, 1).reshape(G * w, 3))
    return np.concatenate(parts, 0)


def kernel(x, W, b, c_rho, c_p, c_u):
    x = np.asarray(x, np.float32)
    nc = get_nc()
    w1, w2 = pack_weights(W, b, c_rho, c_p, c_u)

    in_maps = []
    for c in range(N_CORES):
        off = c * PER_CORE_RAW
        xc = np.zeros((PER_CORE, 3), np.float32)
        xc[:PER_CORE_RAW] = x[off:off + PER_CORE_RAW]
        in_maps.append({"x4": pack_x_core(xc), "w1": w1, "w2": w2})

    res = run_bass_kernel_spmd(nc, in_maps, list(range(N_CORES)),
                               trace=PROFILE)
    if PROFILE:
        globals()["LAST_RESULT"] = res
    outs = []
    for c in range(N_CORES):
        outs.append(unpack_out_core(res.results[c]["out"])[:PER_CORE_RAW])
    full = np.concatenate(outs, axis=0)  # [1M, 3]
    return (np.ascontiguousarray(full[:, 0:1]),
            np.ascontiguousarray(full[:, 1:2]),
            np.ascontiguousarray(full[:, 2:3]))
